# revision 22
# baseline (speedup 1.0000x reference)
"""Trainium2 Bass kernel for deformable attention.

Contract: kernel(**inputs) takes the FULL inputs (as produced by the problem's
setup_inputs) and returns the FULL [4, 1024, 256] float32 output. Internally the
work is sharded over 8 NeuronCores: core c handles batch c//2 and query half
c%2 (512 queries), with the batch's full value feature map replicated on the
core.

Per-core pipeline (all shapes hardcoded for B=4, Q=1024, D=256, H=W=128,
nh=8, npts=4):
  1. The value projection W_v commutes past the (linear) bilinear/attention
     reduce, so it is folded into the output projection on the host:
     Wcomb_h = W_v @ W_out_h and bvW_h = b_v @ W_out_h, with a per-(q,h)
     sum-of-weights term correcting the bias at zero-padded borders. The
     kernel therefore gathers raw bf16 value rows -- no feature-map GEMM.
  2. Coefficient chain: offsets/attention GEMMs + softmax + bilinear weight
     computation, in [query-partition, sample-free] layout, fp32. Per q-tile
     of 128 queries; gather indices for a tile ship as soon as they finish.
     Everything the q-tile-0 index path needs (transposed query tile,
     pre-scaled W_off, reference points, broadcast b_off) arrives in ONE
     host-packed header DMA so the first gather launches ~10us in; the 0.1
     offset scale and exp(b_attn) softmax bias are folded on the host.
  3. Gather indices are moved into the SWDGE layout ([16 partitions
     replicated x8, (pair, qq, idx-group) free]) without DMA: a DVE
     broadcast copy replicates each 16-wide index group 8x along the free
     dim, an f32 PE transpose flips it across all 128 partitions in one
     shot, and a strided DVE copy (f32->i16) drops it into the interleaved
     free layout the descriptor generator reads.
  4. Gather: per (query, head, point, row-corner) descriptor, one dma_gather
     element of 512 bf16 values = two adjacent columns at one row of the
     value map (overlapping row-pair access pattern; 1024 idxs per call --
     larger calls crash the hardware).
  5. Weighted reduce on the TensorEngine: the 128 gathered slots of a query
     pair are the contraction dim (gathered tile is the stationary operand);
     the moving operand is a masked block-diagonal [128, 16] weight matrix
     built from bilinear*attention weights. Output lands as [d, (q, h)] in
     PSUM, which is exactly the lhsT layout the final GEMM needs.
  6. out = weighted @ Wcomb + sw * bvW + b_out. Query tiles 0-2 and rows
     384..479 run in the normal orientation overlapped with the gather
     stream; the last 32 rows run flipped (weights stationary, queries
     moving) in two 16-row pieces whose raw [ch, q] result ships via a
     second output tensor that the host transposes, so the post-gather
     tail is just a quarter-size reduce plus one short GEMM and store.
"""

from contextlib import ExitStack

import numpy as np
import ml_dtypes

NH, NPTS = 8, 4
D = 256
HW = 128            # H == W == 128
NROWS = HW * HW     # 16384
QPC = 512           # queries per core
NCORES = 8
NPAIRS = QPC // 2   # 256 query pairs
NCHUNK = 32         # gather chunks (>1024 idxs per dma_gather crashes HW)
PAIRS_PER_CHUNK = NPAIRS // NCHUNK  # 8
IDX_PER_CHUNK = PAIRS_PER_CHUNK * 128  # 1024

_CACHE = {}


def _mask16_np():
    """[128, 16] bf16: mask[qq*64 + h*8 + p*2 + yp, qq*8 + h] = 1."""
    m = np.zeros((128, 16), dtype=np.float32)
    for qq in range(2):
        for h in range(NH):
            for p in range(NPTS):
                for yp in range(2):
                    m[qq * 64 + h * 8 + p * 2 + yp, qq * 8 + h] = 1.0
    return m.astype(ml_dtypes.bfloat16)


def _build_bass():
    import concourse.bass as bass
    import concourse.bacc as bacc
    import concourse.mybir as mybir
    import concourse.tile as tile
    from concourse.masks import make_identity

    f32 = mybir.dt.float32
    bf16 = mybir.dt.bfloat16
    i16 = mybir.dt.int16
    i32 = mybir.dt.int32
    Alu = mybir.AluOpType
    Act = mybir.ActivationFunctionType

    nc = bacc.Bacc("TRN2", target_bir_lowering=False,
                   dynamic_dma_scratch_size=32768)

    # ---- I/O ----
    # hdr: host-packed ramp-critical inputs for query tile 0:
    #   [0:256]   qT tile 0 (two 128-col halves)
    #   [256:384] W_off * 0.1 in [p, t, n] layout (two 64-col halves)
    #   [384:386] reference points for queries 0..127
    #   [386:450] b_off * 0.1 (broadcast over partitions)
    hdr = nc.dram_tensor("hdr", [128, 450], f32, kind="ExternalInput")
    query = nc.dram_tensor("query", [D, QPC], f32, kind="ExternalInput")
    refp = nc.dram_tensor("reference_points", [QPC, 2], f32, kind="ExternalInput")
    value = nc.dram_tensor("value", [NROWS, D], bf16, kind="ExternalInput")
    W_attn = nc.dram_tensor("W_attn", [D, 32], f32, kind="ExternalInput")
    b_attn = nc.dram_tensor("b_attn", [128, 32], f32, kind="ExternalInput")
    Wcomb = nc.dram_tensor("Wcomb", [NH * D, D], bf16, kind="ExternalInput")
    bvW = nc.dram_tensor("bvW", [NH, D], bf16, kind="ExternalInput")
    b_out = nc.dram_tensor("b_out", [D], f32, kind="ExternalInput")
    out = nc.dram_tensor("out", [QPC, D], f32, kind="ExternalOutput")
    # rows 480..511 ship in raw [ch%128, ch//128, half, q] layout; the host
    # transposes them during assembly (saves PE transposes in the tail)
    out_tail = nc.dram_tensor("out_tail", [128, 2, 2, 16], f32, kind="ExternalOutput")

    mask_dram = nc.inline_tensor(_mask16_np(), name="mask16")

    with tile.TileContext(nc) as tc, ExitStack() as ctx:
        sb = ctx.enter_context(tc.tile_pool(name="sb", bufs=1))
        ps = ctx.enter_context(tc.tile_pool(name="ps", bufs=1, space="PSUM"))

        # ---- input loads: one packed header DMA carries everything the
        # qtile-0 index path needs; bulk loads ride later HWDGE slots.
        hdr_sb = sb.tile([128, 450], f32, tag="hdr")
        nc.sync.dma_start(hdr_sb[:], hdr[:])
        wcat_at = sb.tile([128, 2, 32], f32, tag="wcat_at")
        nc.sync.dma_start(wcat_at[:], W_attn[:].rearrange("(t p) n -> p t n", p=128))
        eb_bc = sb.tile([128, 8, 4], f32, tag="eb_bc")
        nc.sync.dma_start(eb_bc[:].rearrange("p a b -> p (a b)"), b_attn[:])
        qTrest = sb.tile([128, 2, 384], f32, tag="qTrest")
        nc.sync.dma_start(qTrest[:], query[:].rearrange("(t p) q -> p t q", p=128)[:, :, 128:512])
        rprest = sb.tile([128, 3, 2], f32, tag="rprest")
        nc.sync.dma_start(rprest[:], refp[:].rearrange("(t p) c -> p t c", p=128)[:, 1:4, :])
        mask16 = sb.tile([128, 16], bf16, tag="mask16")
        wout_bf = sb.tile([128, 16, 256], bf16, tag="wout")
        with tc.tile_wait_until(0.0045):
            nc.scalar.dma_start(wout_bf[:], Wcomb[:].rearrange("(t p) n -> p t n", p=128))
        bvw_bf = sb.tile([8, 256], bf16, tag="bvw")
        bout_sb = sb.tile([1, 256], f32, tag="bout")
        with tc.tile_wait_until(0.012):
            nc.scalar.dma_start(mask16[:], mask_dram[:])
            nc.scalar.dma_start(bvw_bf[:], bvW[:])
            nc.scalar.dma_start(bout_sb[:], b_out[None, :])

        # per-qtile views of the packed header / rest tensors
        def qT_slice(qt, t):
            if qt == 0:
                return hdr_sb[:, t * 128:(t + 1) * 128]
            return qTrest[:, t, (qt - 1) * 128:qt * 128]

        def rp_view(qt):
            if qt == 0:
                return hdr_sb[:, 384:386]
            return rprest[:, qt - 1, :]

        wcat_off = [hdr_sb[:, 256:320].rearrange("p (t n) -> p t n", t=1),
                    hdr_sb[:, 320:384]]
        boff_v = hdr_sb[:, 386:450].rearrange("p (a b) -> p a b", b=2)

        # ---- constants built on-chip ----
        ident = sb.tile([128, 128], f32, tag="ident")
        make_identity(nc, ident[:])
        ones1 = sb.tile([1, 128], f32, tag="ones1")
        nc.vector.memset(ones1[:], 1.0)

        # persistent intermediates
        w_a_i = sb.tile([128, 256], bf16, tag="w_a_i")    # [(qq,s64), pair]
        w_b_i = sb.tile([128, 256], bf16, tag="w_b_i")
        # gather indices in SWDGE layout: [16-part replicated x8, qt, j, qq, g4]
        idxt = sb.tile([128, 4, 64, 2, 4], i16, tag="idxt")
        red = sb.tile([128, 2, 512, 8], bf16, tag="red")  # [dlo, dh, q, h]
        swT = sb.tile([8, 512], bf16, tag="swT")          # sum of weights [h, q]
        wabT = sb.tile([128, 512], f32, tag="wabT")       # [(AB,h,p,yp), q]

        # ================= coefficient phase (4 q-tiles of 128) =============
        # pass 1: offsets GEMM + gather-index path per q-tile (ships indices
        # as early as possible); pass 2 below computes the weights.
        P1_WAIT_MS = [None, 0.010, 0.011, 0.012]
        P2_WAIT_MS = [0.006, 0.012, 0.014, 0.016]
        qt_state = []
        for qt in range(4):
          with tc.tile_wait_until(P1_WAIT_MS[qt] or 0,
                                  enable=P1_WAIT_MS[qt] is not None):
            # rpb = broadcast b_off + reference point: independent of the GEMM
            rpb = sb.tile([128, 32, 2], f32, tag="rpb", bufs=4)
            nc.vector.tensor_tensor(
                rpb[:], boff_v,
                rp_view(qt)[:, None, :].to_broadcast([128, 32, 2]), Alu.add)

            psc = ps.tile([128, 64], f32, tag="tp", bufs=2)
            nc.tensor.matmul(psc[:], qT_slice(qt, 0), hdr_sb[:, 256:320], start=True, stop=False)
            nc.tensor.matmul(psc[:], qT_slice(qt, 1), hdr_sb[:, 320:384], start=False, stop=True)

            # sampling grid -> pixel coords, x/y interleaved [128, 32, 2]
            t_u = sb.tile([128, 32, 2], f32, tag="t_u", bufs=4)
            nc.vector.tensor_tensor(
                t_u[:], psc[:].rearrange("p (s c) -> p s c", c=2),
                rpb[:], Alu.add)
            t_c = t_u  # in-place ok per-element
            nc.vector.tensor_scalar(t_c[:], t_u[:], 0.0, 1.0, Alu.max, Alu.min)
            pxs = sb.tile([128, 64], f32, tag="pxs", bufs=4)  # px + 128
            nc.vector.tensor_scalar(pxs[:], t_c[:].rearrange("p a b -> p (a b)"),
                                    128.0, 127.5, Alu.mult, Alu.add)
            ri = sb.tile([128, 64], i32, tag="ri", bufs=4)
            nc.vector.tensor_copy(ri[:], pxs[:])
            rf = sb.tile([128, 64], f32, tag="rf", bufs=4)
            nc.vector.tensor_copy(rf[:], ri[:])
            gt = sb.tile([128, 64], f32, tag="gt", bufs=4)
            nc.vector.tensor_tensor(gt[:], rf[:], pxs[:], Alu.is_gt)
            flr = sb.tile([128, 64], f32, tag="flr", bufs=4)  # floor(px) + 128
            nc.vector.tensor_tensor(flr[:], rf[:], gt[:], Alu.subtract)
            st = sb.tile([128, 64], f32, tag="st", bufs=4)    # clip start + 128
            nc.vector.tensor_scalar(st[:], flr[:], 128.0, 254.0, Alu.max, Alu.min)
            # ---- gather-index path first: this q-tile's gathers can start
            # while the weight path below is still computing ----
            tbase = sb.tile([128, 32], f32, tag="tbase", bufs=4)
            nc.vector.tensor_scalar(
                tbase[:], st[:].rearrange("p (s c) -> p s c", c=2)[:, :, 1],
                128.0, -16512.0, Alu.mult, Alu.add)
            idx64 = sb.tile([128, 32, 2], f32, tag="idx64", bufs=4)
            nc.vector.tensor_tensor(idx64[:, :, 0], tbase[:],
                                    st[:].rearrange("p (s c) -> p s c", c=2)[:, :, 0], Alu.add)
            nc.vector.tensor_scalar_add(idx64[:, :, 1], idx64[:, :, 0], 128.0)
            # replicate 8x along free dim (f32 -> i16), then one int16
            # transpose per 16-wide group lands all 128 partitions at once.
            r16 = sb.tile([128, 4, 8, 16], f32, tag="r16", bufs=4)
            nc.vector.tensor_copy(
                r16[:],
                idx64[:].rearrange("p s c -> p (s c)")
                .rearrange("p (g r) -> p g r", g=4)[:, :, None, :]
                .to_broadcast([128, 4, 8, 16]))
            for g4 in range(4):
                pidx = ps.tile([128, 128], f32, tag="pidx", bufs=2)
                nc.tensor.transpose(
                    pidx[:], r16[:, g4].rearrange("p a b -> p (a b)"), ident[:])
                nc.vector.tensor_copy(
                    idxt[:, qt, :, :, g4],
                    pidx[:].rearrange("p (j q) -> p j q", q=2))
            qt_state.append((qt, pxs, flr, st))

        # ---- weight paths for all q-tiles (can trail into the gather phase;
        # only the reduce matmuls consume the weights) ----
        for qt in range(4):
          with tc.tile_wait_until(P2_WAIT_MS[qt]):
            qtv, pxs, flr, st = qt_state[qt]
            jsl = slice(qt * 64, (qt + 1) * 64)
            # attention logits GEMM + softmax + bilinear weights
            psat = ps.tile([128, 32], f32, tag="pidx", bufs=2)
            nc.tensor.matmul(psat[:], qT_slice(qtv, 0), wcat_at[:, 0, :], start=True, stop=False)
            nc.tensor.matmul(psat[:], qT_slice(qtv, 1), wcat_at[:, 1, :], start=False, stop=True)
            coef = sb.tile([128, 32], f32, tag="coef_sb", bufs=4)
            nc.vector.tensor_copy(coef[:], psat[:])
            ew0 = sb.tile([128, 8, 4], f32, tag="ew0", bufs=4)
            nc.scalar.activation(ew0[:], coef[:], Act.Exp)
            expw = sb.tile([128, 8, 4], f32, tag="expw", bufs=4)
            nc.vector.tensor_tensor(expw[:], ew0[:], eb_bc[:], Alu.mult)
            den = sb.tile([128, 8], f32, tag="den", bufs=4)
            nc.vector.tensor_reduce(den[:], expw[:], axis=mybir.AxisListType.X, op=Alu.add)
            rden = sb.tile([128, 8], f32, tag="rden", bufs=4)
            nc.vector.reciprocal(rden[:], den[:])
            attn = sb.tile([128, 32], f32, tag="attn", bufs=4)
            nc.vector.tensor_tensor(
                attn[:].rearrange("p (h f) -> p h f", f=4), expw[:],
                rden[:, :, None].to_broadcast([128, 8, 4]), Alu.mult)
            w1 = sb.tile([128, 64], f32, tag="w1", bufs=4)
            nc.vector.tensor_tensor(w1[:], pxs[:], flr[:], Alu.subtract)
            dd = sb.tile([128, 64], f32, tag="dd", bufs=4)
            nc.vector.tensor_tensor(dd[:], flr[:], st[:], Alu.subtract)
            m0 = sb.tile([128, 64], f32, tag="m0", bufs=4)
            nc.vector.tensor_scalar(m0[:], dd[:], 0.0, None, Alu.is_equal)
            mneg = sb.tile([128, 64], f32, tag="mneg", bufs=4)
            nc.vector.tensor_scalar(mneg[:], dd[:], -1.0, None, Alu.is_equal)
            mpos = sb.tile([128, 64], f32, tag="mpos", bufs=4)
            nc.vector.tensor_scalar(mpos[:], dd[:], 1.0, None, Alu.is_equal)
            u0 = sb.tile([128, 64], f32, tag="u0", bufs=4)
            nc.vector.tensor_scalar(u0[:], w1[:], 1.0, -1.0, Alu.subtract, Alu.mult)
            # wA = u0*m0 + u1*mneg ; wB = u1*m0 + u0*mpos   (u1 == w1)
            tA = sb.tile([128, 64], f32, tag="tA", bufs=4)
            nc.vector.tensor_tensor(tA[:], u0[:], m0[:], Alu.mult)
            tB = sb.tile([128, 64], f32, tag="tB", bufs=4)
            nc.vector.tensor_tensor(tB[:], w1[:], mneg[:], Alu.mult)
            wA = sb.tile([128, 32, 2], f32, tag="wA", bufs=4)
            nc.vector.tensor_tensor(wA[:].rearrange("p a b -> p (a b)"), tA[:], tB[:], Alu.add)
            nc.vector.tensor_tensor(tA[:], w1[:], m0[:], Alu.mult)
            nc.vector.tensor_tensor(tB[:], u0[:], mpos[:], Alu.mult)
            wB = sb.tile([128, 32, 2], f32, tag="wB", bufs=4)
            nc.vector.tensor_tensor(wB[:].rearrange("p a b -> p (a b)"), tA[:], tB[:], Alu.add)

            # combine with attention; build wab [128, (AB, h*p, yp)]
            aw = sb.tile([128, 32], f32, tag="aw", bufs=4)
            nc.vector.tensor_tensor(aw[:], attn[:], wA[:, :, 0], Alu.mult)
            bw = sb.tile([128, 32], f32, tag="bw", bufs=4)
            nc.vector.tensor_tensor(bw[:], attn[:], wB[:, :, 0], Alu.mult)
            vcat = sb.tile([128, 32, 2], f32, tag="vcat", bufs=4)
            nc.vector.tensor_copy(vcat[:, :, 0], wA[:, :, 1])
            nc.vector.tensor_copy(vcat[:, :, 1], wB[:, :, 1])
            wab = sb.tile([128, 2, 32, 2], f32, tag="wab", bufs=4)
            nc.vector.tensor_tensor(wab[:, 0], vcat[:],
                                    aw[:, :, None].to_broadcast([128, 32, 2]), Alu.mult)
            nc.vector.tensor_tensor(wab[:, 1], vcat[:],
                                    bw[:, :, None].to_broadcast([128, 32, 2]), Alu.mult)

            # sum of all weights per (q, h) -- border-clip correction for the
            # folded b_v term: sw = sum_{AB,p,yp} wab
            swq = sb.tile([128, 8], f32, tag="swq", bufs=4)
            nc.vector.tensor_reduce(
                swq[:], wab[:].rearrange("p a (h r) c -> p h a r c", h=8),
                axis=mybir.AxisListType.XYZ, op=Alu.add)

            pst2b = ps.tile([128, 256], f32, tag="tp", bufs=2)
            nc.tensor.transpose(pst2b[:, 0:128],
                                wab[:].rearrange("p a s c -> p (a s c)"), ident[:])
            pst3 = ps.tile([8, 128], f32, tag="img", bufs=2)
            nc.tensor.transpose(pst3[:], swq[:], ident[:])
            nc.vector.tensor_copy(wabT[:, qt * 128:(qt + 1) * 128], pst2b[:, 0:128])
            nc.vector.tensor_copy(swT[:, qt * 128:(qt + 1) * 128], pst3[:])
            nc.vector.tensor_copy(w_a_i[0:64, jsl], wabT[0:64, qt * 128:qt * 128 + 128:2])
            nc.vector.tensor_copy(w_a_i[64:128, jsl], wabT[0:64, qt * 128 + 1:qt * 128 + 128:2])
            nc.vector.tensor_copy(w_b_i[0:64, jsl], wabT[64:128, qt * 128:qt * 128 + 128:2])
            nc.vector.tensor_copy(w_b_i[64:128, jsl], wabT[64:128, qt * 128 + 1:qt * 128 + 128:2])

        # ================= gather + weighted reduce =========================
        import concourse.bass as bass_mod
        gather_src = bass_mod.AP(
            tensor=value, offset=0, ap=[[256, NROWS - 1], [1, 512]])

        # chunk list in pairs: 31 full chunks of 8, then two of 4 so the
        # post-gather tail only waits on a quarter-size reduce.
        chunk_bounds = [(0, 4), (4, 8)] + [(g * 8, g * 8 + 8) for g in range(1, 31)] + [(248, 254), (254, 256)]
        for p0, p1 in chunk_bounds:
            npair = p1 - p0
            qt, j0 = p0 // 64, p0 % 64
            gsl = slice(p0, p1)
            gt_sb = sb.tile([128, npair, 512], bf16, tag="gat", bufs=12)
            nc.gpsimd.dma_gather(
                out_ap=gt_sb[:],
                in_ap=gather_src,
                idxs_ap=idxt[:, qt, j0:j0 + npair, :, :].rearrange(
                    "p a b c -> p (a b c)"),
                num_idxs=npair * 128,
                num_idxs_reg=npair * 128,
                elem_size=512,
                elem_step=256,
            )
            wblkA = sb.tile([128, npair, 16], bf16, tag="wblkA", bufs=4)
            nc.vector.tensor_tensor(
                wblkA[:], mask16[:, None, :].to_broadcast([128, npair, 16]),
                w_a_i[:, gsl, None].to_broadcast([128, npair, 16]), Alu.mult)
            wblkB = sb.tile([128, npair, 16], bf16, tag="wblkB", bufs=4)
            nc.vector.tensor_tensor(
                wblkB[:], mask16[:, None, :].to_broadcast([128, npair, 16]),
                w_b_i[:, gsl, None].to_broadcast([128, npair, 16]), Alu.mult)

            last = (p0, p1) == chunk_bounds[-1]
            plo = ps.tile([128, npair * 16], f32, tag="tp" if last else "red_lo", bufs=2 if last else 1)
            phi = ps.tile([128, npair * 16], f32, tag="pidx" if last else "red_hi", bufs=2 if last else 1)
            for j in range(npair):
                osl = slice(j * 16, (j + 1) * 16)
                nc.tensor.matmul(plo[:, osl], gt_sb[:, j, 0:128], wblkA[:, j, :],
                                 start=True, stop=False)
                nc.tensor.matmul(phi[:, osl], gt_sb[:, j, 128:256], wblkA[:, j, :],
                                 start=True, stop=False)
                nc.tensor.matmul(plo[:, osl], gt_sb[:, j, 256:384], wblkB[:, j, :],
                                 start=False, stop=True)
                nc.tensor.matmul(phi[:, osl], gt_sb[:, j, 384:512], wblkB[:, j, :],
                                 start=False, stop=True)
            qsl2 = slice(p0 * 2, p1 * 2)
            nc.vector.tensor_copy(
                red[:, 0, qsl2, :].rearrange("p a b -> p (a b)"), plo[:])
            nc.vector.tensor_copy(
                red[:, 1, qsl2, :].rearrange("p a b -> p (a b)"), phi[:])

        # ================= out = weighted @ W_out + b_out ===================
        # qtiles 0-2 and rows 384..479 in the normal orientation (these all
        # complete during the gather stream); the last 32 rows flipped so the
        # post-gather tail is just 34 short matmuls + transposes.
        for qt in range(3):
            pso = ps.tile([128, 256], f32, tag="img", bufs=2)
            for kt in range(16):
                h, dh = kt // 2, kt % 2
                lhsT = red[:, dh, qt * 128:(qt + 1) * 128, h]
                nc.tensor.matmul(pso[:], lhsT, wout_bf[:, kt, :],
                                 start=(kt == 0), stop=False)
            nc.tensor.matmul(pso[:], swT[:, qt * 128:(qt + 1) * 128], bvw_bf[:],
                             start=False, stop=False)
            nc.tensor.matmul(pso[:], ones1[:], bout_sb[:], start=False, stop=True)
            o_sb = sb.tile([128, 256], f32, tag="o_sb", bufs=2)
            nc.vector.tensor_copy(o_sb[:], pso[:])
            nc.sync.dma_start(out[qt * 128:(qt + 1) * 128, :], o_sb[:])

        # rows 384..479 (chunks 24-29): normal orientation
        psa = ps.tile([96, 256], f32, tag="img", bufs=2)
        for kt in range(16):
            h, dh = kt // 2, kt % 2
            nc.tensor.matmul(psa[:], red[:, dh, 384:480, h], wout_bf[:, kt, :],
                             start=(kt == 0), stop=False)
        nc.tensor.matmul(psa[:], swT[:, 384:480], bvw_bf[:], start=False, stop=False)
        nc.tensor.matmul(psa[:], ones1[:, 0:96], bout_sb[:], start=False, stop=True)
        o_sa = sb.tile([96, 256], f32, tag="o_sb", bufs=2)
        nc.vector.tensor_copy(o_sa[:], psa[:])
        nc.sync.dma_start(out[384:480, :], o_sa[:])

        # rows 480..511 (chunks 30-31): flipped, Wcomb stationary
        for half, (r0, r1) in enumerate([(480, 496), (496, 512)]):
            pf = ps.tile([128, 2, 16], f32, tag="tp", bufs=2)
            for kt in range(16):
                h, dh = kt // 2, kt % 2
                rhs = red[:, dh, r0:r1, h]
                nc.tensor.matmul(pf[:, 0, :], wout_bf[:, kt, 0:128], rhs,
                                 start=(kt == 0), stop=False, skip_group_check=True)
                nc.tensor.matmul(pf[:, 1, :], wout_bf[:, kt, 128:256], rhs,
                                 start=False, stop=False, skip_group_check=True)
            nc.tensor.matmul(pf[:, 0, :], bvw_bf[:, 0:128], swT[:, r0:r1],
                             start=False, stop=False, skip_group_check=True)
            nc.tensor.matmul(pf[:, 1, :], bvw_bf[:, 128:256], swT[:, r0:r1],
                             start=False, stop=False, skip_group_check=True)
            nc.tensor.matmul(pf[:, 0, :], bout_sb[:, 0:128], ones1[:, 0:16],
                             start=False, stop=False, skip_group_check=True)
            nc.tensor.matmul(pf[:, 1, :], bout_sb[:, 128:256], ones1[:, 0:16],
                             start=False, stop=True, skip_group_check=True)
            f_sb = sb.tile([128, 2, 16], f32, tag="f_sb", bufs=2)
            nc.vector.tensor_copy(f_sb[:], pf[:])
            nc.sync.dma_start(out_tail[:, :, half, :], f_sb[:])

    nc.compile()
    return nc


def _get_nc():
    if "nc" not in _CACHE:
        _CACHE["nc"] = _build_bass()
    return _CACHE["nc"]


def _make_in_maps(inputs):
    query = np.ascontiguousarray(np.asarray(inputs["query"], dtype=np.float32))
    refp = np.ascontiguousarray(np.asarray(inputs["reference_points"], dtype=np.float32))
    value = np.ascontiguousarray(
        np.asarray(inputs["value"], dtype=np.float32).astype(ml_dtypes.bfloat16))
    consts = {
        k: np.ascontiguousarray(np.asarray(inputs[k], np.float32))
        for k in ["W_attn", "b_out"]
    }
    consts["b_attn"] = np.ascontiguousarray(np.broadcast_to(
        np.exp(np.asarray(inputs["b_attn"], np.float64)).astype(np.float32),
        (128, 32)).copy())
    W_off_s = np.asarray(inputs["W_off"], np.float32) * 0.1
    b_off_s = np.asarray(inputs["b_off"], np.float32) * 0.1
    W_v = np.asarray(inputs["W_v"], np.float64)
    b_v = np.asarray(inputs["b_v"], np.float64)
    W_out = np.asarray(inputs["W_out"], np.float64).reshape(NH, D, D)
    consts["Wcomb"] = np.ascontiguousarray(
        np.einsum("ij,hjk->hik", W_v, W_out).reshape(NH * D, D).astype(ml_dtypes.bfloat16))
    consts["bvW"] = np.ascontiguousarray(
        np.einsum("j,hjk->hk", b_v, W_out).astype(ml_dtypes.bfloat16))
    in_maps = []
    for c in range(NCORES):
        b, s = c // 2, c % 2
        qsl = slice(s * QPC, (s + 1) * QPC)
        qc = query[b, qsl]
        rc = refp[b, qsl]
        hdr = np.empty((128, 450), np.float32)
        hdr[:, 0:128] = qc[0:128, 0:128].T
        hdr[:, 128:256] = qc[0:128, 128:256].T
        hdr[:, 256:320] = W_off_s[0:128, :]
        hdr[:, 320:384] = W_off_s[128:256, :]
        hdr[:, 384:386] = rc[0:128, :]
        hdr[:, 386:450] = np.broadcast_to(b_off_s, (128, 64))
        in_maps.append({
            "hdr": np.ascontiguousarray(hdr),
            "query": np.ascontiguousarray(qc.T),
            "reference_points": np.ascontiguousarray(rc),
            "value": np.ascontiguousarray(value[b]),
            **consts,
        })
    return in_maps


def _assemble(outs, shape):
    out = np.zeros(shape, dtype=np.float32)
    for c in range(NCORES):
        b, s = c // 2, c % 2
        out[b, s * QPC:(s + 1) * QPC] = outs[c]["out"]
        tail = np.asarray(outs[c]["out_tail"])  # [ch%128, ch//128, half, q]
        tail = tail.transpose(2, 3, 1, 0).reshape(32, 256)  # [half*q, ch]
        out[b, s * QPC + 480:(s + 1) * QPC] = tail
    return out


def kernel(query, reference_points, value, W_off, b_off, W_attn, b_attn,
           W_v, b_v, W_out, b_out, H=128, W=128, **_unused):
    assert int(H) == HW and int(W) == HW
    from concourse.bass_utils import run_bass_kernel_spmd

    inputs = dict(query=query, reference_points=reference_points, value=value,
                  W_off=W_off, b_off=b_off, W_attn=W_attn, b_attn=b_attn,
                  W_v=W_v, b_v=b_v, W_out=W_out, b_out=b_out)
    in_maps = _make_in_maps(inputs)
    nc = _get_nc()
    res = run_bass_kernel_spmd(nc, in_maps, core_ids=list(range(NCORES)))
    outs = res.results if hasattr(res, "results") else res
    B, Q, _ = np.asarray(query).shape
    return _assemble(outs, (B, Q, D))


# revision 23
# speedup vs baseline: 1.0014x; 1.0014x over previous
"""Trainium2 Bass kernel for deformable attention.

Contract: kernel(**inputs) takes the FULL inputs (as produced by the problem's
setup_inputs) and returns the FULL [4, 1024, 256] float32 output. Internally the
work is sharded over 8 NeuronCores: core c handles batch c//2 and query half
c%2 (512 queries), with the batch's full value feature map replicated on the
core.

Per-core pipeline (all shapes hardcoded for B=4, Q=1024, D=256, H=W=128,
nh=8, npts=4):
  1. The value projection W_v commutes past the (linear) bilinear/attention
     reduce, so it is folded into the output projection on the host:
     Wcomb_h = W_v @ W_out_h and bvW_h = b_v @ W_out_h, with a per-(q,h)
     sum-of-weights term correcting the bias at zero-padded borders. The
     kernel therefore gathers raw bf16 value rows -- no feature-map GEMM.
  2. Coefficient chain: offsets/attention GEMMs + softmax + bilinear weight
     computation, in [query-partition, sample-free] layout, fp32. Per q-tile
     of 128 queries; gather indices for a tile ship as soon as they finish.
     Everything the q-tile-0 index path needs (transposed query tile,
     pre-scaled W_off, reference points, broadcast b_off) arrives in ONE
     host-packed header DMA so the first gather launches ~10us in; the 0.1
     offset scale and exp(b_attn) softmax bias are folded on the host.
  3. Gather indices are moved into the SWDGE layout ([16 partitions
     replicated x8, (pair, qq, idx-group) free]) without DMA: a DVE
     broadcast copy replicates each 16-wide index group 8x along the free
     dim, an f32 PE transpose flips it across all 128 partitions in one
     shot, and a strided DVE copy (f32->i16) drops it into the interleaved
     free layout the descriptor generator reads.
  4. Gather: per (query, head, point, row-corner) descriptor, one dma_gather
     element of 512 bf16 values = two adjacent columns at one row of the
     value map (overlapping row-pair access pattern; 1024 idxs per call --
     larger calls crash the hardware).
  5. Weighted reduce on the TensorEngine: the 128 gathered slots of a query
     pair are the contraction dim (gathered tile is the stationary operand);
     the moving operand is a masked block-diagonal [128, 16] weight matrix
     built from bilinear*attention weights. Output lands as [d, (q, h)] in
     PSUM, which is exactly the lhsT layout the final GEMM needs.
  6. out = weighted @ Wcomb + sw * bvW + b_out. Query tiles 0-2 and rows
     384..479 run in the normal orientation overlapped with the gather
     stream; the last 32 rows run flipped (weights stationary, queries
     moving) in two 16-row pieces whose raw [ch, q] result ships via a
     second output tensor that the host transposes, so the post-gather
     tail is just a quarter-size reduce plus one short GEMM and store.
"""

from contextlib import ExitStack

import numpy as np
import ml_dtypes

NH, NPTS = 8, 4
D = 256
HW = 128            # H == W == 128
NROWS = HW * HW     # 16384
QPC = 512           # queries per core
NCORES = 8
NPAIRS = QPC // 2   # 256 query pairs
NCHUNK = 32         # gather chunks (>1024 idxs per dma_gather crashes HW)
PAIRS_PER_CHUNK = NPAIRS // NCHUNK  # 8
IDX_PER_CHUNK = PAIRS_PER_CHUNK * 128  # 1024

_CACHE = {}


def _mask16_np():
    """[128, 16] bf16: mask[qq*64 + h*8 + p*2 + yp, qq*8 + h] = 1."""
    m = np.zeros((128, 16), dtype=np.float32)
    for qq in range(2):
        for h in range(NH):
            for p in range(NPTS):
                for yp in range(2):
                    m[qq * 64 + h * 8 + p * 2 + yp, qq * 8 + h] = 1.0
    return m.astype(ml_dtypes.bfloat16)


def _build_bass():
    import concourse.bass as bass
    import concourse.bacc as bacc
    import concourse.mybir as mybir
    import concourse.tile as tile
    from concourse.masks import make_identity

    f32 = mybir.dt.float32
    bf16 = mybir.dt.bfloat16
    i16 = mybir.dt.int16
    i32 = mybir.dt.int32
    Alu = mybir.AluOpType
    Act = mybir.ActivationFunctionType

    nc = bacc.Bacc("TRN2", target_bir_lowering=False,
                   dynamic_dma_scratch_size=32768)

    # ---- I/O ----
    # hdr: host-packed ramp-critical inputs for query tile 0:
    #   [0:256]   qT tile 0 (two 128-col halves)
    #   [256:384] W_off * 0.1 in [p, t, n] layout (two 64-col halves)
    #   [384:386] reference points for queries 0..127
    #   [386:450] b_off * 0.1 (broadcast over partitions)
    hdr = nc.dram_tensor("hdr", [128, 450], f32, kind="ExternalInput")
    query = nc.dram_tensor("query", [D, QPC], f32, kind="ExternalInput")
    refp = nc.dram_tensor("reference_points", [QPC, 2], f32, kind="ExternalInput")
    value = nc.dram_tensor("value", [NROWS, D], bf16, kind="ExternalInput")
    W_attn = nc.dram_tensor("W_attn", [D, 32], f32, kind="ExternalInput")
    b_attn = nc.dram_tensor("b_attn", [128, 32], f32, kind="ExternalInput")
    Wcomb = nc.dram_tensor("Wcomb", [NH * D, D], bf16, kind="ExternalInput")
    bvW = nc.dram_tensor("bvW", [NH, D], bf16, kind="ExternalInput")
    b_out = nc.dram_tensor("b_out", [D], f32, kind="ExternalInput")
    out = nc.dram_tensor("out", [QPC, D], f32, kind="ExternalOutput")
    # rows 480..511 ship in raw [ch%128, ch//128, half, q] layout; the host
    # transposes them during assembly (saves PE transposes in the tail)
    out_tail = nc.dram_tensor("out_tail", [128, 2, 2, 16], f32, kind="ExternalOutput")

    mask_dram = nc.inline_tensor(_mask16_np(), name="mask16")

    with tile.TileContext(nc) as tc, ExitStack() as ctx:
        sb = ctx.enter_context(tc.tile_pool(name="sb", bufs=1))
        ps = ctx.enter_context(tc.tile_pool(name="ps", bufs=1, space="PSUM"))

        # ---- input loads: one packed header DMA carries everything the
        # qtile-0 index path needs; bulk loads ride later HWDGE slots.
        hdr_sb = sb.tile([128, 450], f32, tag="hdr")
        nc.sync.dma_start(hdr_sb[:], hdr[:])
        wcat_at = sb.tile([128, 2, 32], f32, tag="wcat_at")
        nc.sync.dma_start(wcat_at[:], W_attn[:].rearrange("(t p) n -> p t n", p=128))
        eb_bc = sb.tile([128, 8, 4], f32, tag="eb_bc")
        nc.sync.dma_start(eb_bc[:].rearrange("p a b -> p (a b)"), b_attn[:])
        qTrest = sb.tile([128, 2, 384], f32, tag="qTrest")
        nc.sync.dma_start(qTrest[:], query[:].rearrange("(t p) q -> p t q", p=128)[:, :, 128:512])
        rprest = sb.tile([128, 3, 2], f32, tag="rprest")
        nc.sync.dma_start(rprest[:], refp[:].rearrange("(t p) c -> p t c", p=128)[:, 1:4, :])
        mask16 = sb.tile([128, 16], bf16, tag="mask16")
        wout_bf = sb.tile([128, 16, 256], bf16, tag="wout")
        with tc.tile_wait_until(0.0045):
            nc.scalar.dma_start(wout_bf[:], Wcomb[:].rearrange("(t p) n -> p t n", p=128))
        bvw_bf = sb.tile([8, 256], bf16, tag="bvw")
        bout_sb = sb.tile([1, 256], f32, tag="bout")
        with tc.tile_wait_until(0.0075):
            nc.scalar.dma_start(mask16[:], mask_dram[:])
            nc.scalar.dma_start(bvw_bf[:], bvW[:])
            nc.scalar.dma_start(bout_sb[:], b_out[None, :])

        # per-qtile views of the packed header / rest tensors
        def qT_slice(qt, t):
            if qt == 0:
                return hdr_sb[:, t * 128:(t + 1) * 128]
            return qTrest[:, t, (qt - 1) * 128:qt * 128]

        def rp_view(qt):
            if qt == 0:
                return hdr_sb[:, 384:386]
            return rprest[:, qt - 1, :]

        wcat_off = [hdr_sb[:, 256:320].rearrange("p (t n) -> p t n", t=1),
                    hdr_sb[:, 320:384]]
        boff_v = hdr_sb[:, 386:450].rearrange("p (a b) -> p a b", b=2)

        # ---- constants built on-chip ----
        ident = sb.tile([128, 128], f32, tag="ident")
        make_identity(nc, ident[:])
        ones1 = sb.tile([1, 128], f32, tag="ones1")
        nc.vector.memset(ones1[:], 1.0)

        # persistent intermediates
        w_a_i = sb.tile([128, 256], bf16, tag="w_a_i")    # [(qq,s64), pair]
        w_b_i = sb.tile([128, 256], bf16, tag="w_b_i")
        # gather indices in SWDGE layout: [16-part replicated x8, qt, j, qq, g4]
        idxt = sb.tile([128, 4, 64, 2, 4], i16, tag="idxt")
        red = sb.tile([128, 2, 512, 8], bf16, tag="red")  # [dlo, dh, q, h]
        swT = sb.tile([8, 512], bf16, tag="swT")          # sum of weights [h, q]
        wabT = sb.tile([128, 512], f32, tag="wabT")       # [(AB,h,p,yp), q]

        # ================= coefficient phase (4 q-tiles of 128) =============
        # pass 1: offsets GEMM + gather-index path per q-tile (ships indices
        # as early as possible); pass 2 below computes the weights.
        P1_WAIT_MS = [None, 0.010, 0.011, 0.012]
        P2_WAIT_MS = [0.006, 0.012, 0.014, 0.016]
        qt_state = []
        for qt in range(4):
          with tc.tile_wait_until(P1_WAIT_MS[qt] or 0,
                                  enable=P1_WAIT_MS[qt] is not None):
            # rpb = broadcast b_off + reference point: independent of the GEMM
            rpb = sb.tile([128, 32, 2], f32, tag="rpb", bufs=4)
            nc.vector.tensor_tensor(
                rpb[:], boff_v,
                rp_view(qt)[:, None, :].to_broadcast([128, 32, 2]), Alu.add)

            psc = ps.tile([128, 64], f32, tag="tp", bufs=2)
            nc.tensor.matmul(psc[:], qT_slice(qt, 0), hdr_sb[:, 256:320], start=True, stop=False)
            nc.tensor.matmul(psc[:], qT_slice(qt, 1), hdr_sb[:, 320:384], start=False, stop=True)

            # sampling grid -> pixel coords, x/y interleaved [128, 32, 2]
            t_u = sb.tile([128, 32, 2], f32, tag="t_u", bufs=4)
            nc.vector.tensor_tensor(
                t_u[:], psc[:].rearrange("p (s c) -> p s c", c=2),
                rpb[:], Alu.add)
            t_c = t_u  # in-place ok per-element
            nc.vector.tensor_scalar(t_c[:], t_u[:], 0.0, 1.0, Alu.max, Alu.min)
            pxs = sb.tile([128, 64], f32, tag="pxs", bufs=4)  # px + 128
            nc.vector.tensor_scalar(pxs[:], t_c[:].rearrange("p a b -> p (a b)"),
                                    128.0, 127.5, Alu.mult, Alu.add)
            ri = sb.tile([128, 64], i32, tag="ri", bufs=4)
            nc.vector.tensor_copy(ri[:], pxs[:])
            rf = sb.tile([128, 64], f32, tag="rf", bufs=4)
            nc.vector.tensor_copy(rf[:], ri[:])
            gt = sb.tile([128, 64], f32, tag="gt", bufs=4)
            nc.vector.tensor_tensor(gt[:], rf[:], pxs[:], Alu.is_gt)
            flr = sb.tile([128, 64], f32, tag="flr", bufs=4)  # floor(px) + 128
            nc.vector.tensor_tensor(flr[:], rf[:], gt[:], Alu.subtract)
            st = sb.tile([128, 64], f32, tag="st", bufs=4)    # clip start + 128
            nc.vector.tensor_scalar(st[:], flr[:], 128.0, 254.0, Alu.max, Alu.min)
            # ---- gather-index path first: this q-tile's gathers can start
            # while the weight path below is still computing ----
            tbase = sb.tile([128, 32], f32, tag="tbase", bufs=4)
            nc.vector.tensor_scalar(
                tbase[:], st[:].rearrange("p (s c) -> p s c", c=2)[:, :, 1],
                128.0, -16512.0, Alu.mult, Alu.add)
            idx64 = sb.tile([128, 32, 2], f32, tag="idx64", bufs=4)
            nc.vector.tensor_tensor(idx64[:, :, 0], tbase[:],
                                    st[:].rearrange("p (s c) -> p s c", c=2)[:, :, 0], Alu.add)
            nc.vector.tensor_scalar_add(idx64[:, :, 1], idx64[:, :, 0], 128.0)
            # replicate 8x along free dim (f32 -> i16), then one int16
            # transpose per 16-wide group lands all 128 partitions at once.
            r16 = sb.tile([128, 4, 8, 16], f32, tag="r16", bufs=4)
            idx64g = (idx64[:].rearrange("p s c -> p (s c)")
                      .rearrange("p (g r) -> p g r", g=4))
            for g4 in range(4):
                nc.vector.tensor_copy(
                    r16[:, g4],
                    idx64g[:, g4, None, :].to_broadcast([128, 8, 16]))
                pidx = ps.tile([128, 128], f32, tag="pidx", bufs=2)
                nc.tensor.transpose(
                    pidx[:], r16[:, g4].rearrange("p a b -> p (a b)"), ident[:])
                nc.vector.tensor_copy(
                    idxt[:, qt, :, :, g4],
                    pidx[:].rearrange("p (j q) -> p j q", q=2))
            qt_state.append((qt, pxs, flr, st))

        # ---- weight paths for all q-tiles (can trail into the gather phase;
        # only the reduce matmuls consume the weights) ----
        for qt in range(4):
          with tc.tile_wait_until(P2_WAIT_MS[qt]):
            qtv, pxs, flr, st = qt_state[qt]
            jsl = slice(qt * 64, (qt + 1) * 64)
            # attention logits GEMM + softmax + bilinear weights
            psat = ps.tile([128, 32], f32, tag="pidx", bufs=2)
            nc.tensor.matmul(psat[:], qT_slice(qtv, 0), wcat_at[:, 0, :], start=True, stop=False)
            nc.tensor.matmul(psat[:], qT_slice(qtv, 1), wcat_at[:, 1, :], start=False, stop=True)
            coef = sb.tile([128, 32], f32, tag="coef_sb", bufs=4)
            nc.vector.tensor_copy(coef[:], psat[:])
            ew0 = sb.tile([128, 8, 4], f32, tag="ew0", bufs=4)
            nc.scalar.activation(ew0[:], coef[:], Act.Exp)
            expw = sb.tile([128, 8, 4], f32, tag="expw", bufs=4)
            nc.vector.tensor_tensor(expw[:], ew0[:], eb_bc[:], Alu.mult)
            den = sb.tile([128, 8], f32, tag="den", bufs=4)
            nc.vector.tensor_reduce(den[:], expw[:], axis=mybir.AxisListType.X, op=Alu.add)
            rden = sb.tile([128, 8], f32, tag="rden", bufs=4)
            nc.vector.reciprocal(rden[:], den[:])
            attn = sb.tile([128, 32], f32, tag="attn", bufs=4)
            nc.vector.tensor_tensor(
                attn[:].rearrange("p (h f) -> p h f", f=4), expw[:],
                rden[:, :, None].to_broadcast([128, 8, 4]), Alu.mult)
            w1 = sb.tile([128, 64], f32, tag="w1", bufs=4)
            nc.vector.tensor_tensor(w1[:], pxs[:], flr[:], Alu.subtract)
            dd = sb.tile([128, 64], f32, tag="dd", bufs=4)
            nc.vector.tensor_tensor(dd[:], flr[:], st[:], Alu.subtract)
            m0 = sb.tile([128, 64], f32, tag="m0", bufs=4)
            nc.vector.tensor_scalar(m0[:], dd[:], 0.0, None, Alu.is_equal)
            mneg = sb.tile([128, 64], f32, tag="mneg", bufs=4)
            nc.vector.tensor_scalar(mneg[:], dd[:], -1.0, None, Alu.is_equal)
            mpos = sb.tile([128, 64], f32, tag="mpos", bufs=4)
            nc.vector.tensor_scalar(mpos[:], dd[:], 1.0, None, Alu.is_equal)
            u0 = sb.tile([128, 64], f32, tag="u0", bufs=4)
            nc.vector.tensor_scalar(u0[:], w1[:], 1.0, -1.0, Alu.subtract, Alu.mult)
            # wA = u0*m0 + u1*mneg ; wB = u1*m0 + u0*mpos   (u1 == w1)
            tA = sb.tile([128, 64], f32, tag="tA", bufs=4)
            nc.vector.tensor_tensor(tA[:], u0[:], m0[:], Alu.mult)
            tB = sb.tile([128, 64], f32, tag="tB", bufs=4)
            nc.vector.tensor_tensor(tB[:], w1[:], mneg[:], Alu.mult)
            wA = sb.tile([128, 32, 2], f32, tag="wA", bufs=4)
            nc.vector.tensor_tensor(wA[:].rearrange("p a b -> p (a b)"), tA[:], tB[:], Alu.add)
            nc.vector.tensor_tensor(tA[:], w1[:], m0[:], Alu.mult)
            nc.vector.tensor_tensor(tB[:], u0[:], mpos[:], Alu.mult)
            wB = sb.tile([128, 32, 2], f32, tag="wB", bufs=4)
            nc.vector.tensor_tensor(wB[:].rearrange("p a b -> p (a b)"), tA[:], tB[:], Alu.add)

            # combine with attention; build wab [128, (AB, h*p, yp)]
            aw = sb.tile([128, 32], f32, tag="aw", bufs=4)
            nc.vector.tensor_tensor(aw[:], attn[:], wA[:, :, 0], Alu.mult)
            bw = sb.tile([128, 32], f32, tag="bw", bufs=4)
            nc.vector.tensor_tensor(bw[:], attn[:], wB[:, :, 0], Alu.mult)
            vcat = sb.tile([128, 32, 2], f32, tag="vcat", bufs=4)
            nc.vector.tensor_copy(vcat[:, :, 0], wA[:, :, 1])
            nc.vector.tensor_copy(vcat[:, :, 1], wB[:, :, 1])
            wab = sb.tile([128, 2, 32, 2], f32, tag="wab", bufs=4)
            nc.vector.tensor_tensor(wab[:, 0], vcat[:],
                                    aw[:, :, None].to_broadcast([128, 32, 2]), Alu.mult)
            nc.vector.tensor_tensor(wab[:, 1], vcat[:],
                                    bw[:, :, None].to_broadcast([128, 32, 2]), Alu.mult)

            # sum of all weights per (q, h) -- border-clip correction for the
            # folded b_v term: sw = sum_{AB,p,yp} wab
            swq = sb.tile([128, 8], f32, tag="swq", bufs=4)
            nc.vector.tensor_reduce(
                swq[:], wab[:].rearrange("p a (h r) c -> p h a r c", h=8),
                axis=mybir.AxisListType.XYZ, op=Alu.add)

            pst2b = ps.tile([128, 256], f32, tag="tp", bufs=2)
            nc.tensor.transpose(pst2b[:, 0:128],
                                wab[:].rearrange("p a s c -> p (a s c)"), ident[:])
            pst3 = ps.tile([8, 128], f32, tag="img", bufs=2)
            nc.tensor.transpose(pst3[:], swq[:], ident[:])
            nc.vector.tensor_copy(wabT[:, qt * 128:(qt + 1) * 128], pst2b[:, 0:128])
            nc.vector.tensor_copy(swT[:, qt * 128:(qt + 1) * 128], pst3[:])
            nc.vector.tensor_copy(w_a_i[0:64, jsl], wabT[0:64, qt * 128:qt * 128 + 128:2])
            nc.vector.tensor_copy(w_a_i[64:128, jsl], wabT[0:64, qt * 128 + 1:qt * 128 + 128:2])
            nc.vector.tensor_copy(w_b_i[0:64, jsl], wabT[64:128, qt * 128:qt * 128 + 128:2])
            nc.vector.tensor_copy(w_b_i[64:128, jsl], wabT[64:128, qt * 128 + 1:qt * 128 + 128:2])

        # ================= gather + weighted reduce =========================
        import concourse.bass as bass_mod
        gather_src = bass_mod.AP(
            tensor=value, offset=0, ap=[[256, NROWS - 1], [1, 512]])

        # chunk list in pairs: 31 full chunks of 8, then two of 4 so the
        # post-gather tail only waits on a quarter-size reduce.
        chunk_bounds = [(0, 4), (4, 8)] + [(g * 8, g * 8 + 8) for g in range(1, 31)] + [(248, 254), (254, 256)]
        for p0, p1 in chunk_bounds:
            npair = p1 - p0
            qt, j0 = p0 // 64, p0 % 64
            gsl = slice(p0, p1)
            gt_sb = sb.tile([128, npair, 512], bf16, tag="gat", bufs=12)
            nc.gpsimd.dma_gather(
                out_ap=gt_sb[:],
                in_ap=gather_src,
                idxs_ap=idxt[:, qt, j0:j0 + npair, :, :].rearrange(
                    "p a b c -> p (a b c)"),
                num_idxs=npair * 128,
                num_idxs_reg=npair * 128,
                elem_size=512,
                elem_step=256,
            )
            wblkA = sb.tile([128, npair, 16], bf16, tag="wblkA", bufs=4)
            nc.vector.tensor_tensor(
                wblkA[:], mask16[:, None, :].to_broadcast([128, npair, 16]),
                w_a_i[:, gsl, None].to_broadcast([128, npair, 16]), Alu.mult)
            wblkB = sb.tile([128, npair, 16], bf16, tag="wblkB", bufs=4)
            nc.vector.tensor_tensor(
                wblkB[:], mask16[:, None, :].to_broadcast([128, npair, 16]),
                w_b_i[:, gsl, None].to_broadcast([128, npair, 16]), Alu.mult)

            last = (p0, p1) == chunk_bounds[-1]
            plo = ps.tile([128, npair * 16], f32, tag="tp" if last else "red_lo", bufs=2 if last else 1)
            phi = ps.tile([128, npair * 16], f32, tag="pidx" if last else "red_hi", bufs=2 if last else 1)
            for j in range(npair):
                osl = slice(j * 16, (j + 1) * 16)
                nc.tensor.matmul(plo[:, osl], gt_sb[:, j, 0:128], wblkA[:, j, :],
                                 start=True, stop=False)
                nc.tensor.matmul(phi[:, osl], gt_sb[:, j, 128:256], wblkA[:, j, :],
                                 start=True, stop=False)
                nc.tensor.matmul(plo[:, osl], gt_sb[:, j, 256:384], wblkB[:, j, :],
                                 start=False, stop=True)
                nc.tensor.matmul(phi[:, osl], gt_sb[:, j, 384:512], wblkB[:, j, :],
                                 start=False, stop=True)
            qsl2 = slice(p0 * 2, p1 * 2)
            nc.vector.tensor_copy(
                red[:, 0, qsl2, :].rearrange("p a b -> p (a b)"), plo[:])
            nc.vector.tensor_copy(
                red[:, 1, qsl2, :].rearrange("p a b -> p (a b)"), phi[:])

        # ================= out = weighted @ W_out + b_out ===================
        # qtiles 0-2 and rows 384..479 in the normal orientation (these all
        # complete during the gather stream); the last 32 rows flipped so the
        # post-gather tail is just 34 short matmuls + transposes.
        for qt in range(3):
            pso = ps.tile([128, 256], f32, tag="img", bufs=2)
            for kt in range(16):
                h, dh = kt // 2, kt % 2
                lhsT = red[:, dh, qt * 128:(qt + 1) * 128, h]
                nc.tensor.matmul(pso[:], lhsT, wout_bf[:, kt, :],
                                 start=(kt == 0), stop=False)
            nc.tensor.matmul(pso[:], swT[:, qt * 128:(qt + 1) * 128], bvw_bf[:],
                             start=False, stop=False)
            nc.tensor.matmul(pso[:], ones1[:], bout_sb[:], start=False, stop=True)
            o_sb = sb.tile([128, 256], f32, tag="o_sb", bufs=2)
            nc.vector.tensor_copy(o_sb[:], pso[:])
            nc.sync.dma_start(out[qt * 128:(qt + 1) * 128, :], o_sb[:])

        # rows 384..479 (chunks 24-29): normal orientation
        psa = ps.tile([96, 256], f32, tag="img", bufs=2)
        for kt in range(16):
            h, dh = kt // 2, kt % 2
            nc.tensor.matmul(psa[:], red[:, dh, 384:480, h], wout_bf[:, kt, :],
                             start=(kt == 0), stop=False)
        nc.tensor.matmul(psa[:], swT[:, 384:480], bvw_bf[:], start=False, stop=False)
        nc.tensor.matmul(psa[:], ones1[:, 0:96], bout_sb[:], start=False, stop=True)
        o_sa = sb.tile([96, 256], f32, tag="o_sb", bufs=2)
        nc.vector.tensor_copy(o_sa[:], psa[:])
        nc.sync.dma_start(out[384:480, :], o_sa[:])

        # rows 480..511 (chunks 30-31): flipped, Wcomb stationary
        for half, (r0, r1) in enumerate([(480, 496), (496, 512)]):
            pf = ps.tile([128, 2, 16], f32, tag="tp", bufs=2)
            for kt in range(16):
                h, dh = kt // 2, kt % 2
                rhs = red[:, dh, r0:r1, h]
                nc.tensor.matmul(pf[:, 0, :], wout_bf[:, kt, 0:128], rhs,
                                 start=(kt == 0), stop=False, skip_group_check=True)
                nc.tensor.matmul(pf[:, 1, :], wout_bf[:, kt, 128:256], rhs,
                                 start=False, stop=False, skip_group_check=True)
            nc.tensor.matmul(pf[:, 0, :], bvw_bf[:, 0:128], swT[:, r0:r1],
                             start=False, stop=False, skip_group_check=True)
            nc.tensor.matmul(pf[:, 1, :], bvw_bf[:, 128:256], swT[:, r0:r1],
                             start=False, stop=False, skip_group_check=True)
            nc.tensor.matmul(pf[:, 0, :], bout_sb[:, 0:128], ones1[:, 0:16],
                             start=False, stop=False, skip_group_check=True)
            nc.tensor.matmul(pf[:, 1, :], bout_sb[:, 128:256], ones1[:, 0:16],
                             start=False, stop=True, skip_group_check=True)
            f_sb = sb.tile([128, 2, 16], f32, tag="f_sb", bufs=2)
            nc.vector.tensor_copy(f_sb[:], pf[:])
            nc.sync.dma_start(out_tail[:, :, half, :], f_sb[:])

    nc.compile()
    return nc


def _get_nc():
    if "nc" not in _CACHE:
        _CACHE["nc"] = _build_bass()
    return _CACHE["nc"]


def _make_in_maps(inputs):
    query = np.ascontiguousarray(np.asarray(inputs["query"], dtype=np.float32))
    refp = np.ascontiguousarray(np.asarray(inputs["reference_points"], dtype=np.float32))
    value = np.ascontiguousarray(
        np.asarray(inputs["value"], dtype=np.float32).astype(ml_dtypes.bfloat16))
    consts = {
        k: np.ascontiguousarray(np.asarray(inputs[k], np.float32))
        for k in ["W_attn", "b_out"]
    }
    consts["b_attn"] = np.ascontiguousarray(np.broadcast_to(
        np.exp(np.asarray(inputs["b_attn"], np.float64)).astype(np.float32),
        (128, 32)).copy())
    W_off_s = np.asarray(inputs["W_off"], np.float32) * 0.1
    b_off_s = np.asarray(inputs["b_off"], np.float32) * 0.1
    W_v = np.asarray(inputs["W_v"], np.float64)
    b_v = np.asarray(inputs["b_v"], np.float64)
    W_out = np.asarray(inputs["W_out"], np.float64).reshape(NH, D, D)
    consts["Wcomb"] = np.ascontiguousarray(
        np.einsum("ij,hjk->hik", W_v, W_out).reshape(NH * D, D).astype(ml_dtypes.bfloat16))
    consts["bvW"] = np.ascontiguousarray(
        np.einsum("j,hjk->hk", b_v, W_out).astype(ml_dtypes.bfloat16))
    in_maps = []
    for c in range(NCORES):
        b, s = c // 2, c % 2
        qsl = slice(s * QPC, (s + 1) * QPC)
        qc = query[b, qsl]
        rc = refp[b, qsl]
        hdr = np.empty((128, 450), np.float32)
        hdr[:, 0:128] = qc[0:128, 0:128].T
        hdr[:, 128:256] = qc[0:128, 128:256].T
        hdr[:, 256:320] = W_off_s[0:128, :]
        hdr[:, 320:384] = W_off_s[128:256, :]
        hdr[:, 384:386] = rc[0:128, :]
        hdr[:, 386:450] = np.broadcast_to(b_off_s, (128, 64))
        in_maps.append({
            "hdr": np.ascontiguousarray(hdr),
            "query": np.ascontiguousarray(qc.T),
            "reference_points": np.ascontiguousarray(rc),
            "value": np.ascontiguousarray(value[b]),
            **consts,
        })
    return in_maps


def _assemble(outs, shape):
    out = np.zeros(shape, dtype=np.float32)
    for c in range(NCORES):
        b, s = c // 2, c % 2
        out[b, s * QPC:(s + 1) * QPC] = outs[c]["out"]
        tail = np.asarray(outs[c]["out_tail"])  # [ch%128, ch//128, half, q]
        tail = tail.transpose(2, 3, 1, 0).reshape(32, 256)  # [half*q, ch]
        out[b, s * QPC + 480:(s + 1) * QPC] = tail
    return out


def kernel(query, reference_points, value, W_off, b_off, W_attn, b_attn,
           W_v, b_v, W_out, b_out, H=128, W=128, **_unused):
    assert int(H) == HW and int(W) == HW
    from concourse.bass_utils import run_bass_kernel_spmd

    inputs = dict(query=query, reference_points=reference_points, value=value,
                  W_off=W_off, b_off=b_off, W_attn=W_attn, b_attn=b_attn,
                  W_v=W_v, b_v=b_v, W_out=W_out, b_out=b_out)
    in_maps = _make_in_maps(inputs)
    nc = _get_nc()
    res = run_bass_kernel_spmd(nc, in_maps, core_ids=list(range(NCORES)))
    outs = res.results if hasattr(res, "results") else res
    B, Q, _ = np.asarray(query).shape
    return _assemble(outs, (B, Q, D))


# revision 25
# speedup vs baseline: 1.0066x; 1.0051x over previous
"""Trainium2 Bass kernel for deformable attention.

Contract: kernel(**inputs) takes the FULL inputs (as produced by the problem's
setup_inputs) and returns the FULL [4, 1024, 256] float32 output. Internally the
work is sharded over 8 NeuronCores: core c handles batch c//2 and query half
c%2 (512 queries), with the batch's full value feature map replicated on the
core.

Per-core pipeline (all shapes hardcoded for B=4, Q=1024, D=256, H=W=128,
nh=8, npts=4):
  1. The value projection W_v commutes past the (linear) bilinear/attention
     reduce, so it is folded into the output projection on the host:
     Wcomb_h = W_v @ W_out_h and bvW_h = b_v @ W_out_h, with a per-(q,h)
     sum-of-weights term correcting the bias at zero-padded borders. The
     kernel therefore gathers raw bf16 value rows -- no feature-map GEMM.
  2. Coefficient chain: offsets/attention GEMMs + softmax + bilinear weight
     computation, in [query-partition, sample-free] layout, fp32. Per q-tile
     of 128 queries; gather indices for a tile ship as soon as they finish.
     Everything the q-tile-0 index path needs (transposed query tile,
     pre-scaled W_off, reference points, broadcast b_off) arrives in ONE
     host-packed header DMA so the first gather launches ~10us in; the 0.1
     offset scale and exp(b_attn) softmax bias are folded on the host.
  3. Gather indices are moved into the SWDGE layout ([16 partitions
     replicated x8, (pair, qq, idx-group) free]) without DMA: a DVE
     broadcast copy replicates each 16-wide index group 8x along the free
     dim, an f32 PE transpose flips it across all 128 partitions in one
     shot, and a strided DVE copy (f32->i16) drops it into the interleaved
     free layout the descriptor generator reads.
  4. Gather: per (query, head, point, row-corner) descriptor, one dma_gather
     element of 512 bf16 values = two adjacent columns at one row of the
     value map (overlapping row-pair access pattern; 1024 idxs per call --
     larger calls crash the hardware).
  5. Weighted reduce on the TensorEngine: the 128 gathered slots of a query
     pair are the contraction dim (gathered tile is the stationary operand);
     the moving operand is a masked block-diagonal [128, 16] weight matrix
     built from bilinear*attention weights. Output lands as [d, (q, h)] in
     PSUM, which is exactly the lhsT layout the final GEMM needs.
  6. out = weighted @ Wcomb + sw * bvW + b_out. Query tiles 0-2 and rows
     384..479 run in the normal orientation overlapped with the gather
     stream; the last 32 rows run flipped (weights stationary, queries
     moving) in two 16-row pieces whose raw [ch, q] result ships via a
     second output tensor that the host transposes, so the post-gather
     tail is just a quarter-size reduce plus one short GEMM and store.
"""

from contextlib import ExitStack

import numpy as np
import ml_dtypes

NH, NPTS = 8, 4
D = 256
HW = 128            # H == W == 128
NROWS = HW * HW     # 16384
QPC = 512           # queries per core
NCORES = 8
NPAIRS = QPC // 2   # 256 query pairs
NCHUNK = 32         # gather chunks (>1024 idxs per dma_gather crashes HW)
PAIRS_PER_CHUNK = NPAIRS // NCHUNK  # 8
IDX_PER_CHUNK = PAIRS_PER_CHUNK * 128  # 1024

_CACHE = {}


def _mask16_np():
    """[128, 16] bf16: mask[qq*64 + h*8 + p*2 + yp, qq*8 + h] = 1."""
    m = np.zeros((128, 16), dtype=np.float32)
    for qq in range(2):
        for h in range(NH):
            for p in range(NPTS):
                for yp in range(2):
                    m[qq * 64 + h * 8 + p * 2 + yp, qq * 8 + h] = 1.0
    return m.astype(ml_dtypes.bfloat16)


def _build_bass():
    import concourse.bass as bass
    import concourse.bacc as bacc
    import concourse.mybir as mybir
    import concourse.tile as tile
    from concourse.masks import make_identity

    f32 = mybir.dt.float32
    bf16 = mybir.dt.bfloat16
    i16 = mybir.dt.int16
    i32 = mybir.dt.int32
    Alu = mybir.AluOpType
    Act = mybir.ActivationFunctionType

    nc = bacc.Bacc("TRN2", target_bir_lowering=False,
                   dynamic_dma_scratch_size=32768)

    # ---- I/O ----
    # hdr: host-packed ramp-critical inputs for query tile 0:
    #   [0:256]   qT tile 0 (two 128-col halves)
    #   [256:384] W_off * 0.1 in [p, t, n] layout (two 64-col halves)
    #   [384:386] reference points for queries 0..127
    #   [386:450] b_off * 0.1 (broadcast over partitions)
    hdr = nc.dram_tensor("hdr", [128, 450], f32, kind="ExternalInput")
    query = nc.dram_tensor("query", [D, QPC], f32, kind="ExternalInput")
    refp = nc.dram_tensor("reference_points", [QPC, 2], f32, kind="ExternalInput")
    value = nc.dram_tensor("value", [NROWS, D], bf16, kind="ExternalInput")
    W_attn = nc.dram_tensor("W_attn", [D, 32], f32, kind="ExternalInput")
    b_attn = nc.dram_tensor("b_attn", [128, 32], f32, kind="ExternalInput")
    Wcomb = nc.dram_tensor("Wcomb", [NH * D, D], bf16, kind="ExternalInput")
    bvW = nc.dram_tensor("bvW", [NH, D], bf16, kind="ExternalInput")
    b_out = nc.dram_tensor("b_out", [D], f32, kind="ExternalInput")
    out = nc.dram_tensor("out", [QPC, D], f32, kind="ExternalOutput")
    # rows 480..511 ship in raw [ch%128, ch//128, half, q] layout; the host
    # transposes them during assembly (saves PE transposes in the tail)
    out_tail = nc.dram_tensor("out_tail", [128, 2, 2, 16], f32, kind="ExternalOutput")

    mask_dram = nc.inline_tensor(_mask16_np(), name="mask16")

    with tile.TileContext(nc) as tc, ExitStack() as ctx:
        sb = ctx.enter_context(tc.tile_pool(name="sb", bufs=1))
        ps = ctx.enter_context(tc.tile_pool(name="ps", bufs=1, space="PSUM"))

        # ---- input loads: one packed header DMA carries everything the
        # qtile-0 index path needs; bulk loads ride later HWDGE slots.
        hdr_sb = sb.tile([128, 450], f32, tag="hdr")
        nc.sync.dma_start(hdr_sb[:], hdr[:])
        wcat_at = sb.tile([128, 2, 32], f32, tag="wcat_at")
        nc.sync.dma_start(wcat_at[:], W_attn[:].rearrange("(t p) n -> p t n", p=128))
        eb_bc = sb.tile([128, 8, 4], f32, tag="eb_bc")
        nc.sync.dma_start(eb_bc[:].rearrange("p a b -> p (a b)"), b_attn[:])
        qTrest = sb.tile([128, 2, 384], f32, tag="qTrest")
        nc.sync.dma_start(qTrest[:], query[:].rearrange("(t p) q -> p t q", p=128)[:, :, 128:512])
        rprest = sb.tile([128, 3, 2], f32, tag="rprest")
        nc.sync.dma_start(rprest[:], refp[:].rearrange("(t p) c -> p t c", p=128)[:, 1:4, :])
        mask16 = sb.tile([128, 16], bf16, tag="mask16")
        wout_bf = sb.tile([128, 16, 256], bf16, tag="wout")
        with tc.tile_wait_until(0.0045):
            nc.scalar.dma_start(wout_bf[:], Wcomb[:].rearrange("(t p) n -> p t n", p=128))
        bvw_bf = sb.tile([8, 256], bf16, tag="bvw")
        bout_sb = sb.tile([1, 256], f32, tag="bout")
        with tc.tile_wait_until(0.0075):
            nc.scalar.dma_start(mask16[:], mask_dram[:])
            nc.scalar.dma_start(bvw_bf[:], bvW[:])
            nc.scalar.dma_start(bout_sb[:], b_out[None, :])

        # per-qtile views of the packed header / rest tensors
        def qT_slice(qt, t):
            if qt == 0:
                return hdr_sb[:, t * 128:(t + 1) * 128]
            return qTrest[:, t, (qt - 1) * 128:qt * 128]

        def rp_view(qt):
            if qt == 0:
                return hdr_sb[:, 384:386]
            return rprest[:, qt - 1, :]

        wcat_off = [hdr_sb[:, 256:320].rearrange("p (t n) -> p t n", t=1),
                    hdr_sb[:, 320:384]]
        boff_v = hdr_sb[:, 386:450].rearrange("p (a b) -> p a b", b=2)

        # ---- constants built on-chip ----
        ident = sb.tile([128, 128], f32, tag="ident")
        make_identity(nc, ident[:])
        ones1 = sb.tile([1, 128], f32, tag="ones1")
        nc.vector.memset(ones1[:], 1.0)

        # persistent intermediates
        w_a_i = sb.tile([128, 256], bf16, tag="w_a_i")    # [(qq,s64), pair]
        w_b_i = sb.tile([128, 256], bf16, tag="w_b_i")
        # gather indices in SWDGE layout: [16-part replicated x8, qt, j, qq, g4]
        idxt = sb.tile([128, 4, 64, 2, 4], i16, tag="idxt")
        red = sb.tile([128, 2, 512, 8], bf16, tag="red")  # [dlo, dh, q, h]
        swT = sb.tile([8, 512], bf16, tag="swT")          # sum of weights [h, q]
        wabT = sb.tile([128, 512], f32, tag="wabT")       # [(AB,h,p,yp), q]

        # ================= coefficient phase (4 q-tiles of 128) =============
        # pass 1: offsets GEMM + gather-index path per q-tile (ships indices
        # as early as possible); pass 2 below computes the weights.
        P1_WAIT_MS = [None, 0.010, 0.011, 0.012]
        P2_WAIT_MS = [0.006, 0.012, 0.014, 0.016]
        qt_state = []
        for qt in range(4):
          with tc.tile_wait_until(P1_WAIT_MS[qt] or 0,
                                  enable=P1_WAIT_MS[qt] is not None):
            # rpb = broadcast b_off + reference point: independent of the GEMM
            rpb = sb.tile([128, 32, 2], f32, tag="rpb", bufs=4)
            nc.vector.tensor_tensor(
                rpb[:], boff_v,
                rp_view(qt)[:, None, :].to_broadcast([128, 32, 2]), Alu.add)

            psc = ps.tile([128, 64], f32, tag="tp", bufs=2)
            nc.tensor.matmul(psc[:], qT_slice(qt, 0), hdr_sb[:, 256:320], start=True, stop=False)
            nc.tensor.matmul(psc[:], qT_slice(qt, 1), hdr_sb[:, 320:384], start=False, stop=True)

            # sampling grid -> pixel coords, x/y interleaved [128, 32, 2]
            t_u = sb.tile([128, 32, 2], f32, tag="t_u", bufs=4)
            nc.vector.tensor_tensor(
                t_u[:], psc[:].rearrange("p (s c) -> p s c", c=2),
                rpb[:], Alu.add)
            t_c = t_u  # in-place ok per-element
            nc.vector.tensor_scalar(t_c[:], t_u[:], 0.0, 1.0, Alu.max, Alu.min)
            pxs = sb.tile([128, 64], f32, tag="pxs", bufs=4)  # px + 128
            nc.vector.tensor_scalar(pxs[:], t_c[:].rearrange("p a b -> p (a b)"),
                                    128.0, 127.5, Alu.mult, Alu.add)
            ri = sb.tile([128, 64], i32, tag="ri", bufs=4)
            nc.vector.tensor_copy(ri[:], pxs[:])
            rf = sb.tile([128, 64], f32, tag="rf", bufs=4)
            nc.vector.tensor_copy(rf[:], ri[:])
            gt = sb.tile([128, 64], f32, tag="gt", bufs=4)
            nc.vector.tensor_tensor(gt[:], rf[:], pxs[:], Alu.is_gt)
            flr = sb.tile([128, 64], f32, tag="flr", bufs=4)  # floor(px) + 128
            nc.vector.tensor_tensor(flr[:], rf[:], gt[:], Alu.subtract)
            st = sb.tile([128, 64], f32, tag="st", bufs=4)    # clip start + 128
            nc.vector.tensor_scalar(st[:], flr[:], 128.0, 254.0, Alu.max, Alu.min)
            # ---- gather-index path first: this q-tile's gathers can start
            # while the weight path below is still computing ----
            tbase = sb.tile([128, 32], f32, tag="tbase", bufs=4)
            nc.vector.tensor_scalar(
                tbase[:], st[:].rearrange("p (s c) -> p s c", c=2)[:, :, 1],
                128.0, -16512.0, Alu.mult, Alu.add)
            idx64 = sb.tile([128, 32, 2], f32, tag="idx64", bufs=4)
            nc.vector.tensor_tensor(idx64[:, :, 0], tbase[:],
                                    st[:].rearrange("p (s c) -> p s c", c=2)[:, :, 0], Alu.add)
            nc.vector.tensor_scalar_add(idx64[:, :, 1], idx64[:, :, 0], 128.0)
            # replicate 8x along free dim (f32 -> i16), then one int16
            # transpose per 16-wide group lands all 128 partitions at once.
            r16 = sb.tile([128, 4, 8, 16], f32, tag="r16", bufs=4)
            idx64g = (idx64[:].rearrange("p s c -> p (s c)")
                      .rearrange("p (g r) -> p g r", g=4))
            pidx_tiles = []
            for g4 in range(4):
                nc.vector.tensor_copy(
                    r16[:, g4],
                    idx64g[:, g4, None, :].to_broadcast([128, 8, 16]))
                tag = "img" if (qt == 0 and g4 < 2) else "pidx"
                pidx = ps.tile([128, 128], f32, tag=tag, bufs=2)
                nc.tensor.transpose(
                    pidx[:], r16[:, g4].rearrange("p a b -> p (a b)"), ident[:])
                pidx_tiles.append(pidx)
                pv = pidx[:].rearrange("p (j q) -> p j q", q=2)
                if qt == 0:
                    nc.vector.tensor_copy(idxt[:, qt, 0:4, :, g4], pv[:, 0:4, :])
                else:
                    nc.vector.tensor_copy(idxt[:, qt, :, :, g4], pv)
            if qt == 0:
                for g4 in range(4):
                    pv = pidx_tiles[g4][:].rearrange("p (j q) -> p j q", q=2)
                    nc.vector.tensor_copy(idxt[:, qt, 4:64, :, g4], pv[:, 4:64, :])
            qt_state.append((qt, pxs, flr, st))

        # ---- weight paths for all q-tiles (can trail into the gather phase;
        # only the reduce matmuls consume the weights) ----
        for qt in range(4):
          with tc.tile_wait_until(P2_WAIT_MS[qt]):
            qtv, pxs, flr, st = qt_state[qt]
            jsl = slice(qt * 64, (qt + 1) * 64)
            # attention logits GEMM + softmax + bilinear weights
            psat = ps.tile([128, 32], f32, tag="pidx", bufs=2)
            nc.tensor.matmul(psat[:], qT_slice(qtv, 0), wcat_at[:, 0, :], start=True, stop=False)
            nc.tensor.matmul(psat[:], qT_slice(qtv, 1), wcat_at[:, 1, :], start=False, stop=True)
            coef = sb.tile([128, 32], f32, tag="coef_sb", bufs=4)
            nc.vector.tensor_copy(coef[:], psat[:])
            ew0 = sb.tile([128, 8, 4], f32, tag="ew0", bufs=4)
            nc.scalar.activation(ew0[:], coef[:], Act.Exp)
            expw = sb.tile([128, 8, 4], f32, tag="expw", bufs=4)
            nc.vector.tensor_tensor(expw[:], ew0[:], eb_bc[:], Alu.mult)
            den = sb.tile([128, 8], f32, tag="den", bufs=4)
            nc.vector.tensor_reduce(den[:], expw[:], axis=mybir.AxisListType.X, op=Alu.add)
            rden = sb.tile([128, 8], f32, tag="rden", bufs=4)
            nc.vector.reciprocal(rden[:], den[:])
            attn = sb.tile([128, 32], f32, tag="attn", bufs=4)
            nc.vector.tensor_tensor(
                attn[:].rearrange("p (h f) -> p h f", f=4), expw[:],
                rden[:, :, None].to_broadcast([128, 8, 4]), Alu.mult)
            w1 = sb.tile([128, 64], f32, tag="w1", bufs=4)
            nc.vector.tensor_tensor(w1[:], pxs[:], flr[:], Alu.subtract)
            dd = sb.tile([128, 64], f32, tag="dd", bufs=4)
            nc.vector.tensor_tensor(dd[:], flr[:], st[:], Alu.subtract)
            m0 = sb.tile([128, 64], f32, tag="m0", bufs=4)
            nc.vector.tensor_scalar(m0[:], dd[:], 0.0, None, Alu.is_equal)
            mneg = sb.tile([128, 64], f32, tag="mneg", bufs=4)
            nc.vector.tensor_scalar(mneg[:], dd[:], -1.0, None, Alu.is_equal)
            mpos = sb.tile([128, 64], f32, tag="mpos", bufs=4)
            nc.vector.tensor_scalar(mpos[:], dd[:], 1.0, None, Alu.is_equal)
            u0 = sb.tile([128, 64], f32, tag="u0", bufs=4)
            nc.vector.tensor_scalar(u0[:], w1[:], 1.0, -1.0, Alu.subtract, Alu.mult)
            # wA = u0*m0 + u1*mneg ; wB = u1*m0 + u0*mpos   (u1 == w1)
            tA = sb.tile([128, 64], f32, tag="tA", bufs=4)
            nc.vector.tensor_tensor(tA[:], u0[:], m0[:], Alu.mult)
            tB = sb.tile([128, 64], f32, tag="tB", bufs=4)
            nc.vector.tensor_tensor(tB[:], w1[:], mneg[:], Alu.mult)
            wA = sb.tile([128, 32, 2], f32, tag="wA", bufs=4)
            nc.vector.tensor_tensor(wA[:].rearrange("p a b -> p (a b)"), tA[:], tB[:], Alu.add)
            nc.vector.tensor_tensor(tA[:], w1[:], m0[:], Alu.mult)
            nc.vector.tensor_tensor(tB[:], u0[:], mpos[:], Alu.mult)
            wB = sb.tile([128, 32, 2], f32, tag="wB", bufs=4)
            nc.vector.tensor_tensor(wB[:].rearrange("p a b -> p (a b)"), tA[:], tB[:], Alu.add)

            # combine with attention; build wab [128, (AB, h*p, yp)]
            aw = sb.tile([128, 32], f32, tag="aw", bufs=4)
            nc.vector.tensor_tensor(aw[:], attn[:], wA[:, :, 0], Alu.mult)
            bw = sb.tile([128, 32], f32, tag="bw", bufs=4)
            nc.vector.tensor_tensor(bw[:], attn[:], wB[:, :, 0], Alu.mult)
            vcat = sb.tile([128, 32, 2], f32, tag="vcat", bufs=4)
            nc.vector.tensor_copy(vcat[:, :, 0], wA[:, :, 1])
            nc.vector.tensor_copy(vcat[:, :, 1], wB[:, :, 1])
            wab = sb.tile([128, 2, 32, 2], f32, tag="wab", bufs=4)
            nc.vector.tensor_tensor(wab[:, 0], vcat[:],
                                    aw[:, :, None].to_broadcast([128, 32, 2]), Alu.mult)
            nc.vector.tensor_tensor(wab[:, 1], vcat[:],
                                    bw[:, :, None].to_broadcast([128, 32, 2]), Alu.mult)

            # sum of all weights per (q, h) -- border-clip correction for the
            # folded b_v term: sw = sum_{AB,p,yp} wab
            swq = sb.tile([128, 8], f32, tag="swq", bufs=4)
            nc.vector.tensor_reduce(
                swq[:], wab[:].rearrange("p a (h r) c -> p h a r c", h=8),
                axis=mybir.AxisListType.XYZ, op=Alu.add)

            pst2b = ps.tile([128, 256], f32, tag="tp", bufs=2)
            nc.tensor.transpose(pst2b[:, 0:128],
                                wab[:].rearrange("p a s c -> p (a s c)"), ident[:])
            pst3 = ps.tile([8, 128], f32, tag="img", bufs=2)
            nc.tensor.transpose(pst3[:], swq[:], ident[:])
            nc.vector.tensor_copy(wabT[:, qt * 128:(qt + 1) * 128], pst2b[:, 0:128])
            nc.vector.tensor_copy(swT[:, qt * 128:(qt + 1) * 128], pst3[:])
            nc.vector.tensor_copy(w_a_i[0:64, jsl], wabT[0:64, qt * 128:qt * 128 + 128:2])
            nc.vector.tensor_copy(w_a_i[64:128, jsl], wabT[0:64, qt * 128 + 1:qt * 128 + 128:2])
            nc.vector.tensor_copy(w_b_i[0:64, jsl], wabT[64:128, qt * 128:qt * 128 + 128:2])
            nc.vector.tensor_copy(w_b_i[64:128, jsl], wabT[64:128, qt * 128 + 1:qt * 128 + 128:2])

        # ================= gather + weighted reduce =========================
        import concourse.bass as bass_mod
        gather_src = bass_mod.AP(
            tensor=value, offset=0, ap=[[256, NROWS - 1], [1, 512]])

        # chunk list in pairs: 31 full chunks of 8, then two of 4 so the
        # post-gather tail only waits on a quarter-size reduce.
        chunk_bounds = [(0, 4), (4, 8)] + [(g * 8, g * 8 + 8) for g in range(1, 31)] + [(248, 254), (254, 256)]
        for p0, p1 in chunk_bounds:
            npair = p1 - p0
            qt, j0 = p0 // 64, p0 % 64
            gsl = slice(p0, p1)
            gt_sb = sb.tile([128, npair, 512], bf16, tag="gat", bufs=12)
            nc.gpsimd.dma_gather(
                out_ap=gt_sb[:],
                in_ap=gather_src,
                idxs_ap=idxt[:, qt, j0:j0 + npair, :, :].rearrange(
                    "p a b c -> p (a b c)"),
                num_idxs=npair * 128,
                num_idxs_reg=npair * 128,
                elem_size=512,
                elem_step=256,
            )
            wblkA = sb.tile([128, npair, 16], bf16, tag="wblkA", bufs=4)
            nc.vector.tensor_tensor(
                wblkA[:], mask16[:, None, :].to_broadcast([128, npair, 16]),
                w_a_i[:, gsl, None].to_broadcast([128, npair, 16]), Alu.mult)
            wblkB = sb.tile([128, npair, 16], bf16, tag="wblkB", bufs=4)
            nc.vector.tensor_tensor(
                wblkB[:], mask16[:, None, :].to_broadcast([128, npair, 16]),
                w_b_i[:, gsl, None].to_broadcast([128, npair, 16]), Alu.mult)

            last = (p0, p1) == chunk_bounds[-1]
            plo = ps.tile([128, npair * 16], f32, tag="tp" if last else "red_lo", bufs=2 if last else 1)
            phi = ps.tile([128, npair * 16], f32, tag="pidx" if last else "red_hi", bufs=2 if last else 1)
            for j in range(npair):
                osl = slice(j * 16, (j + 1) * 16)
                nc.tensor.matmul(plo[:, osl], gt_sb[:, j, 0:128], wblkA[:, j, :],
                                 start=True, stop=False)
                nc.tensor.matmul(phi[:, osl], gt_sb[:, j, 128:256], wblkA[:, j, :],
                                 start=True, stop=False)
                nc.tensor.matmul(plo[:, osl], gt_sb[:, j, 256:384], wblkB[:, j, :],
                                 start=False, stop=True)
                nc.tensor.matmul(phi[:, osl], gt_sb[:, j, 384:512], wblkB[:, j, :],
                                 start=False, stop=True)
            qsl2 = slice(p0 * 2, p1 * 2)
            nc.vector.tensor_copy(
                red[:, 0, qsl2, :].rearrange("p a b -> p (a b)"), plo[:])
            nc.vector.tensor_copy(
                red[:, 1, qsl2, :].rearrange("p a b -> p (a b)"), phi[:])

        # ================= out = weighted @ W_out + b_out ===================
        # qtiles 0-2 and rows 384..479 in the normal orientation (these all
        # complete during the gather stream); the last 32 rows flipped so the
        # post-gather tail is just 34 short matmuls + transposes.
        for qt in range(3):
            pso = ps.tile([128, 256], f32, tag="img", bufs=2)
            for kt in range(16):
                h, dh = kt // 2, kt % 2
                lhsT = red[:, dh, qt * 128:(qt + 1) * 128, h]
                nc.tensor.matmul(pso[:], lhsT, wout_bf[:, kt, :],
                                 start=(kt == 0), stop=False)
            nc.tensor.matmul(pso[:], swT[:, qt * 128:(qt + 1) * 128], bvw_bf[:],
                             start=False, stop=False)
            nc.tensor.matmul(pso[:], ones1[:], bout_sb[:], start=False, stop=True)
            o_sb = sb.tile([128, 256], f32, tag="o_sb", bufs=2)
            nc.vector.tensor_copy(o_sb[:], pso[:])
            nc.sync.dma_start(out[qt * 128:(qt + 1) * 128, :], o_sb[:])

        # rows 384..479 (chunks 24-29): normal orientation
        psa = ps.tile([96, 256], f32, tag="img", bufs=2)
        for kt in range(16):
            h, dh = kt // 2, kt % 2
            nc.tensor.matmul(psa[:], red[:, dh, 384:480, h], wout_bf[:, kt, :],
                             start=(kt == 0), stop=False)
        nc.tensor.matmul(psa[:], swT[:, 384:480], bvw_bf[:], start=False, stop=False)
        nc.tensor.matmul(psa[:], ones1[:, 0:96], bout_sb[:], start=False, stop=True)
        o_sa = sb.tile([96, 256], f32, tag="o_sb", bufs=2)
        nc.vector.tensor_copy(o_sa[:], psa[:])
        nc.sync.dma_start(out[384:480, :], o_sa[:])

        # rows 480..511 (chunks 30-31): flipped, Wcomb stationary
        for half, (r0, r1) in enumerate([(480, 496), (496, 512)]):
            pf = ps.tile([128, 2, 16], f32, tag="tp", bufs=2)
            for kt in range(16):
                h, dh = kt // 2, kt % 2
                rhs = red[:, dh, r0:r1, h]
                nc.tensor.matmul(pf[:, 0, :], wout_bf[:, kt, 0:128], rhs,
                                 start=(kt == 0), stop=False, skip_group_check=True)
                nc.tensor.matmul(pf[:, 1, :], wout_bf[:, kt, 128:256], rhs,
                                 start=False, stop=False, skip_group_check=True)
            nc.tensor.matmul(pf[:, 0, :], bvw_bf[:, 0:128], swT[:, r0:r1],
                             start=False, stop=False, skip_group_check=True)
            nc.tensor.matmul(pf[:, 1, :], bvw_bf[:, 128:256], swT[:, r0:r1],
                             start=False, stop=False, skip_group_check=True)
            nc.tensor.matmul(pf[:, 0, :], bout_sb[:, 0:128], ones1[:, 0:16],
                             start=False, stop=False, skip_group_check=True)
            nc.tensor.matmul(pf[:, 1, :], bout_sb[:, 128:256], ones1[:, 0:16],
                             start=False, stop=True, skip_group_check=True)
            f_sb = sb.tile([128, 2, 16], f32, tag="f_sb", bufs=2)
            nc.vector.tensor_copy(f_sb[:], pf[:])
            eng = nc.scalar if half == 1 else nc.sync
            eng.dma_start(out_tail[:, :, half, :], f_sb[:])

    nc.compile()
    return nc


def _get_nc():
    if "nc" not in _CACHE:
        _CACHE["nc"] = _build_bass()
    return _CACHE["nc"]


def _make_in_maps(inputs):
    query = np.ascontiguousarray(np.asarray(inputs["query"], dtype=np.float32))
    refp = np.ascontiguousarray(np.asarray(inputs["reference_points"], dtype=np.float32))
    value = np.ascontiguousarray(
        np.asarray(inputs["value"], dtype=np.float32).astype(ml_dtypes.bfloat16))
    consts = {
        k: np.ascontiguousarray(np.asarray(inputs[k], np.float32))
        for k in ["W_attn", "b_out"]
    }
    consts["b_attn"] = np.ascontiguousarray(np.broadcast_to(
        np.exp(np.asarray(inputs["b_attn"], np.float64)).astype(np.float32),
        (128, 32)).copy())
    W_off_s = np.asarray(inputs["W_off"], np.float32) * 0.1
    b_off_s = np.asarray(inputs["b_off"], np.float32) * 0.1
    W_v = np.asarray(inputs["W_v"], np.float64)
    b_v = np.asarray(inputs["b_v"], np.float64)
    W_out = np.asarray(inputs["W_out"], np.float64).reshape(NH, D, D)
    consts["Wcomb"] = np.ascontiguousarray(
        np.einsum("ij,hjk->hik", W_v, W_out).reshape(NH * D, D).astype(ml_dtypes.bfloat16))
    consts["bvW"] = np.ascontiguousarray(
        np.einsum("j,hjk->hk", b_v, W_out).astype(ml_dtypes.bfloat16))
    in_maps = []
    for c in range(NCORES):
        b, s = c // 2, c % 2
        qsl = slice(s * QPC, (s + 1) * QPC)
        qc = query[b, qsl]
        rc = refp[b, qsl]
        hdr = np.empty((128, 450), np.float32)
        hdr[:, 0:128] = qc[0:128, 0:128].T
        hdr[:, 128:256] = qc[0:128, 128:256].T
        hdr[:, 256:320] = W_off_s[0:128, :]
        hdr[:, 320:384] = W_off_s[128:256, :]
        hdr[:, 384:386] = rc[0:128, :]
        hdr[:, 386:450] = np.broadcast_to(b_off_s, (128, 64))
        in_maps.append({
            "hdr": np.ascontiguousarray(hdr),
            "query": np.ascontiguousarray(qc.T),
            "reference_points": np.ascontiguousarray(rc),
            "value": np.ascontiguousarray(value[b]),
            **consts,
        })
    return in_maps


def _assemble(outs, shape):
    out = np.zeros(shape, dtype=np.float32)
    for c in range(NCORES):
        b, s = c // 2, c % 2
        out[b, s * QPC:(s + 1) * QPC] = outs[c]["out"]
        tail = np.asarray(outs[c]["out_tail"])  # [ch%128, ch//128, half, q]
        tail = tail.transpose(2, 3, 1, 0).reshape(32, 256)  # [half*q, ch]
        out[b, s * QPC + 480:(s + 1) * QPC] = tail
    return out


def kernel(query, reference_points, value, W_off, b_off, W_attn, b_attn,
           W_v, b_v, W_out, b_out, H=128, W=128, **_unused):
    assert int(H) == HW and int(W) == HW
    from concourse.bass_utils import run_bass_kernel_spmd

    inputs = dict(query=query, reference_points=reference_points, value=value,
                  W_off=W_off, b_off=b_off, W_attn=W_attn, b_attn=b_attn,
                  W_v=W_v, b_v=b_v, W_out=W_out, b_out=b_out)
    in_maps = _make_in_maps(inputs)
    nc = _get_nc()
    res = run_bass_kernel_spmd(nc, in_maps, core_ids=list(range(NCORES)))
    outs = res.results if hasattr(res, "results") else res
    B, Q, _ = np.asarray(query).shape
    return _assemble(outs, (B, Q, D))
